# revision 3
# baseline (speedup 1.0000x reference)
"""Trainium2 Bass kernel for nn_EncoderLayer_45621142618893 — v2.

Transformer encoder layer (D=1024, H=16 heads, S=2048, B=4), f32 in/out.
8 cores = (batch) x (sequence half); K/V over the full sequence per core
(duplicated across the pair — cheaper than collectives), Q/attention/FFN
for the core's own 1024 tokens. Zero cross-core communication.

v2 vs baseline:
  - ALL inputs packed into one dram tensor (blob) per core.
  - xl (LN1 output) kept SBUF-resident in f32r; no DRAM scratch spill.
  - LN statistic sums + broadcasts done as f32r matmuls (4x cheaper on PE
    than f32).
  - K/Q projections fused per head-pair with attention: only a [128,2048]
    K-pair and [128,1024] Q-pair tile live at a time instead of the full
    K^T/Q^T (saves ~12MB SBUF -> enables xl residency).
  - V projected in bulk (token-major, bf16) from resident xl.
  - Streaming softmax: per-512-chunk max+exp with an exact flash-style
    rescale combine (pipelines PE/DVE/ACT; probs scale runs on GpSimd).
  - FFN W2 slabs double-buffered.

NOTE: f32r-matmul inputs must be PRODUCED as f32r (BIR verifier rule):
x is pre-rounded on host, xl/x2 are written f32r by DVE, k/q by ACT copies.
dma_start_transpose is numerically exact in isolation but corrupted results
in this pipelined attention loop on real hardware — PE transposes are used.

Blob layout [128, 77889] f32 cols (last col = 1.0 for the LN ones-vector):
  0:16384        xT row-blocks [128,2048] x8 (block i at 2048*i)
  16384:24576    wv f32r, V-pass order: two cc-slabs [128, 8, 512] flat
  24576:40960    per-pair [wk_p | wq_p] f32r: pair p at 24576+2048p,
                 each [128, 8, 128] flat 1024 cols (wq pre-scaled by 8)
  40960:77888    bf16 region (bitcast), bf16 col offsets:
      0:8192         wfc slabs j=0..8 of [128, 8, 128] flat
      8192:40960     w1 slabs ht=0..32 of [128, 8, 128] flat
      40960:73728    w2 slabs j=0..8 of [128, 32, 128] flat
      73728:73856    identity [128, 128]
"""

import sys

sys.path.insert(0, "/opt/trn_rl_repo")

import numpy as np
import ml_dtypes

import concourse.bacc as bacc
import concourse.tile as tile
from concourse import mybir
from concourse.bass_utils import run_bass_kernel_spmd

P = 128
D = 1024
S = 2048
SQ = 1024
H = 16
C = 64
FF = 4096
DT = D // P
TT = S // P
FT = FF // P

F32 = mybir.dt.float32
F32R = mybir.dt.float32r
BF16 = mybir.dt.bfloat16

XOFF = 0
WVOFF = 16384
WKQOFF = 24576
BFOFF = 40960
ONE_OFF = 77888
NCOL = 77889
WFC_B = 0
W1_B = 8192
W2_B = 40960
ID_B = 73728

_CACHE = {}


def _build_nc():
    import os
    dbg_stage = os.environ.get("KV2_DEBUG_STAGE", "")
    nc = bacc.Bacc("TRN2", target_bir_lowering=False, debug=False, num_devices=8)

    blob = nc.dram_tensor("blob", [P, NCOL], F32, kind="ExternalInput")
    outT = nc.dram_tensor("outT", [D, SQ], F32, kind="ExternalOutput")
    dbg = None
    if dbg_stage in ("xl", "x2"):
        dbg = nc.dram_tensor("dbg", [P, DT * S if dbg_stage == "xl" else DT * SQ],
                             F32, kind="ExternalOutput")
    elif dbg_stage in ("v", "oT"):
        dbg = nc.dram_tensor("dbg", [P, TT * D if dbg_stage == "v" else DT * SQ],
                             BF16, kind="ExternalOutput")
    elif dbg_stage == "kq":
        dbg = nc.dram_tensor("dbg", [P, S + SQ], F32, kind="ExternalOutput")

    def xrow(i):
        return blob[:, XOFF + S * i: XOFF + S * (i + 1)]

    def xrow_c(j, c0, w):
        return blob[:, XOFF + S * j + c0: XOFF + S * j + c0 + w]

    bfv = blob[:, BFOFF:ONE_OFF].bitcast(BF16)

    with tile.TileContext(nc) as tc:
        cst = tc.alloc_tile_pool(name="cst", bufs=1)
        idt = cst.tile([P, P], BF16, name="idt")
        nc.sync.dma_start(out=idt, in_=bfv[:, ID_B:ID_B + P])
        ones_k = cst.tile([P, 1], F32R, name="ones_k")
        nc.sync.dma_start(out=ones_k, in_=blob[:, ONE_OFF:ONE_OFF + 1].bitcast(F32R))
        ones_m = cst.tile([1, P], F32, name="ones_m")
        nc.vector.memset(ones_m, 1.0)
        eps_t = cst.tile([1, 1], F32, name="eps_t")
        nc.vector.memset(eps_t, 1e-5)

        big = tc.alloc_tile_pool(name="big", bufs=1)
        xl = big.tile([P, DT, S], F32R, name="S1")     # LN1 out, f32r, resident
        v_t = big.tile([P, TT, D], BF16, name="S2")    # V token-major

        # ---------------- Phase 1: LN1 stats + center -> xl ------------------
        with tc.tile_pool(name="p1s", bufs=7) as p1s, \
             tc.tile_pool(name="p1q", bufs=2) as p1q, \
             tc.tile_pool(name="p1r", bufs=1) as p1r, \
             tc.tile_pool(name="p1ps", bufs=1, space="PSUM") as p1ps:
            sx = [p1ps.tile([1, 512], F32, name=f"sx{c}") for c in range(4)]
            sq = [p1ps.tile([1, 512], F32, name=f"sq{c}") for c in range(4)]
            xts = []
            for i in range(DT):
                xt = p1s.tile([P, S], F32R, name="xt")
                xts.append(xt)
                nc.sync.dma_start(out=xt, in_=xrow(i).bitcast(F32R))
                for c in range(4):
                    cs = slice(512 * c, 512 * (c + 1))
                    sqc = p1q.tile([P, 512], F32R, name="sqc")
                    nc.scalar.square(out=sqc, in_=xt[:, cs].bitcast(F32))
                    nc.tensor.matmul(sx[c][:], ones_k[:], xt[:, cs],
                                     start=(i == 0), stop=(i == DT - 1))
                    nc.tensor.matmul(sq[c][:], ones_k[:], sqc[:],
                                     start=(i == 0), stop=(i == DT - 1))
            # 8 tiles in a 7-slot ring: tile 7 evicted block 0's slot, so
            # block 0 must be re-streamed during centering
            xts[0] = None
            mub = p1r.tile([P, S], F32, name="mub")
            rstdb = p1r.tile([P, S], F32, name="rstdb")
            for c in range(4):
                cs = slice(512 * c, 512 * (c + 1))
                mu_c = p1q.tile([1, 512], F32, name="mu_c")
                t_c = p1q.tile([1, 512], F32, name="t_c")
                nc.scalar.mul(out=mu_c, in_=sx[c][:], mul=1.0 / D)
                nc.vector.tensor_mul(t_c, mu_c, mu_c)
                msq_c = p1q.tile([1, 512], F32, name="msq_c")
                nc.scalar.mul(out=msq_c, in_=sq[c][:], mul=1.0 / D)
                nc.vector.tensor_sub(t_c, msq_c, t_c)
                nc.scalar.activation(out=t_c, in_=t_c, func=mybir.ActivationFunctionType.Sqrt,
                                     bias=eps_t, scale=1.0)
                nc.vector.reciprocal(out=t_c, in_=t_c)
                pb = p1ps.tile([P, 512], F32, name=f"sx{c}")
                nc.tensor.matmul(pb[:], ones_m[:], mu_c[:], start=True, stop=True)
                nc.scalar.copy(out=mub[:, cs], in_=pb[:])
                pb2 = p1ps.tile([P, 512], F32, name=f"sq{c}")
                nc.tensor.matmul(pb2[:], ones_m[:], t_c[:], start=True, stop=True)
                nc.scalar.copy(out=rstdb[:, cs], in_=pb2[:])
            # center token-chunk-major so phase 2 can start on early chunks
            for c in range(4):
                cs = slice(512 * c, 512 * (c + 1))
                for i in range(DT):
                    xtc = p1q.tile([P, 512], F32, name="xtc")
                    if xts[i] is None:
                        nc.sync.dma_start(out=xtc, in_=xrow_c(i, 512 * c, 512))
                        nc.vector.tensor_sub(xtc, xtc, mub[:, cs])
                    else:
                        nc.vector.tensor_sub(xtc, xts[i][:, cs].bitcast(F32), mub[:, cs])
                    nc.vector.tensor_mul(xl[:, i, cs], xtc, rstdb[:, cs])

        if dbg_stage == "xl":
            nc.sync.dma_start(out=dbg[:, :], in_=xl.bitcast(F32).rearrange("p a t -> p (a t)"))

        # ---------------- Phase 2: V (bulk, token-major bf16) ----------------
        with tc.tile_pool(name="p2w", bufs=2) as p2w, \
             tc.tile_pool(name="p2ps", bufs=2, space="PSUM") as p2ps:
            for cc in range(2):
                wvs = p2w.tile([P, DT, 512], F32R, name="wvs")
                nc.sync.dma_start(out=wvs, in_=blob[:, WVOFF + 4096 * cc:WVOFF + 4096 * (cc + 1)]
                                  .bitcast(F32R).rearrange("p (a m) -> p a m", a=DT))
                for tt in range(TT):
                    pv = p2ps.tile([P, 512], F32, name="pv")
                    for i in range(DT):
                        nc.tensor.matmul(pv[:], xl[:, i, P * tt:P * (tt + 1)], wvs[:, i, :],
                                         start=(i == 0), stop=(i == DT - 1))
                    nc.scalar.copy(out=v_t[:, tt, 512 * cc:512 * (cc + 1)], in_=pv[:])

        if dbg_stage == "v":
            nc.sync.dma_start(out=dbg[:, :], in_=v_t.rearrange("p a t -> p (a t)"))

        # ------- Phase 3: per head-pair K/Q projection + attention -----------
        oT = big.tile([P, DT, SQ], BF16, name="S3")

        with tc.tile_pool(name="p3w", bufs=2) as p3w, \
             tc.tile_pool(name="p3kq", bufs=3) as p3kq, \
             tc.tile_pool(name="p3a", bufs=3) as p3a, \
             tc.tile_pool(name="p3t", bufs=3) as p3t, \
             tc.tile_pool(name="p3r", bufs=8) as p3r, \
             tc.tile_pool(name="p3ps", bufs=1, space="PSUM") as p3ps, \
             tc.tile_pool(name="p3sc", bufs=4, space="PSUM") as p3sc, \
             tc.tile_pool(name="p3tp", bufs=2, space="PSUM") as p3tp, \
             tc.tile_pool(name="p3ov", bufs=1, space="PSUM") as p3ov:
            for p in range(H // 2):
                wkq = p3w.tile([P, 2, DT, P], F32R, name="wkq")
                nc.sync.dma_start(out=wkq, in_=blob[:, WKQOFF + 2048 * p:WKQOFF + 2048 * (p + 1)]
                                  .bitcast(F32R).rearrange("p (w a m) -> p w a m", w=2, a=DT))
                k_p = p3kq.tile([P, S], F32R, name="k_p")
                for c in range(4):
                    pk = p3ps.tile([P, 512], F32, name="pk")
                    for i in range(DT):
                        nc.tensor.matmul(pk[:], wkq[:, 0, i, :], xl[:, i, 512 * c:512 * (c + 1)],
                                         start=(i == 0), stop=(i == DT - 1))
                    nc.scalar.copy(out=k_p[:, 512 * c:512 * (c + 1)], in_=pk[:])
                q_p = p3kq.tile([P, SQ], F32R, name="q_p")
                for c in range(2):
                    pk = p3ps.tile([P, 512], F32, name="pk")
                    for i in range(DT):
                        nc.tensor.matmul(pk[:], wkq[:, 1, i, :], xl[:, i, 512 * c:512 * (c + 1)],
                                         start=(i == 0), stop=(i == DT - 1))
                    nc.scalar.copy(out=q_p[:, 512 * c:512 * (c + 1)], in_=pk[:])
                if dbg_stage == "kq" and p == 0:
                    nc.sync.dma_start(out=dbg[:, 0:S], in_=k_p[:].bitcast(F32))
                    nc.sync.dma_start(out=dbg[:, S:S + SQ], in_=q_p[:].bitcast(F32))

                for g in range(4):
                    aTs = []
                    for hh in range(2):
                        base = 64 * hh
                        aT = p3t.tile([P, TT, 256], BF16, name="aT")
                        aTs.append(aT)
                        for q2 in range(2):
                            qt = 2 * g + q2
                            at = p3a.tile([P, S], BF16, name="at")
                            mn4 = p3r.tile([P, 4], F32, name="mn4")
                            mmin = p3r.tile([P, 1], F32, name="mmin")
                            f4 = p3r.tile([P, 4], F32, name="f4")
                            zp = p3r.tile([P, 4], F32, name="zp")
                            zz = p3r.tile([P, 4], F32, name="zz")
                            zs = p3r.tile([P, 1], F32, name="zs")
                            rr = p3r.tile([P, 1], F32, name="rr")
                            # streaming softmax: per-512-chunk max+exp, exact
                            # rescale combine at the end (flash-style)
                            for kc in range(4):
                                sck = p3sc.tile([P, 512], F32, name="sc")
                                nc.tensor.matmul(
                                    sck[:],
                                    q_p[base:base + 64, P * qt:P * (qt + 1)],
                                    k_p[base:base + 64, 512 * kc:512 * (kc + 1)],
                                    start=True, stop=True)
                                nc.vector.reduce_max(out=mn4[:, kc:kc + 1], in_=sck[:],
                                                     axis=mybir.AxisListType.X, negate=True)
                                nc.scalar.activation(
                                    out=at[:, 512 * kc:512 * (kc + 1)], in_=sck[:],
                                    func=mybir.ActivationFunctionType.Exp,
                                    bias=mn4[:, kc:kc + 1], scale=1.0,
                                    accum_out=zp[:, kc:kc + 1])
                            # mn4 = -m_kc; global -m = min(mn4); f_kc = exp(m_kc - m)
                            nc.vector.tensor_reduce(out=mmin, in_=mn4, axis=mybir.AxisListType.X,
                                                    op=mybir.AluOpType.min)
                            nc.scalar.activation(out=f4, in_=mn4,
                                                 func=mybir.ActivationFunctionType.Exp,
                                                 bias=mmin, scale=-1.0)
                            nc.vector.tensor_mul(zz, f4, zp)
                            nc.vector.reduce_sum(out=zs, in_=zz, axis=mybir.AxisListType.X)
                            nc.vector.reciprocal(out=rr, in_=zs)
                            nc.vector.tensor_scalar_mul(out=f4, in0=f4, scalar1=rr)
                            for kc in range(4):
                                nc.gpsimd.tensor_scalar_mul(
                                    out=at[:, 512 * kc:512 * (kc + 1)],
                                    in0=at[:, 512 * kc:512 * (kc + 1)],
                                    scalar1=f4[:, kc:kc + 1])
                            for kb in range(4):
                                tp = p3tp.tile([P, 512], BF16, name="tp")
                                for k4 in range(4):
                                    ki = 4 * kb + k4
                                    nc.tensor.transpose(tp[:, P * k4:P * (k4 + 1)],
                                                        at[:, P * ki:P * (ki + 1)], idt[:])
                                dst = aT[:, 4 * kb:4 * (kb + 1), P * q2:P * (q2 + 1)]
                                src = tp.rearrange("p (a b) -> p a b", a=4)
                                if kb % 2 == 0:
                                    nc.vector.tensor_copy(out=dst, in_=src)
                                else:
                                    nc.scalar.copy(out=dst, in_=src)
                    po = p3ov.tile([P, 256], F32, name="po")
                    for hh in range(2):
                        h64 = 64 * (2 * p + hh)
                        for kt in range(TT):
                            nc.tensor.matmul(po[64 * hh:64 * (hh + 1), :], v_t[:, kt, h64:h64 + 64],
                                             aTs[hh][:, kt, :], start=(kt == 0), stop=(kt == TT - 1))
                    nc.scalar.copy(out=oT[:, p, 256 * g:256 * (g + 1)], in_=po[:])

        if dbg_stage == "oT":
            nc.sync.dma_start(out=dbg[:, :], in_=oT.rearrange("p a t -> p (a t)"))

        # ---------------- Phase 4: O-projection + residual -> x2 -------------
        # f32r so LN2's statistic matmuls can consume it directly (the BIR
        # verifier requires f32r-matmul inputs to be produced as f32r)
        x2 = big.tile([P, DT, SQ], F32R, name="S2")

        with tc.tile_pool(name="p4w", bufs=2) as p4w, \
             tc.tile_pool(name="p4x", bufs=2) as p4x, \
             tc.tile_pool(name="p4ps", bufs=3, space="PSUM") as p4ps:
            for j in range(DT):
                ws = p4w.tile([P, DT, P], BF16, name="ws")
                nc.sync.dma_start(out=ws, in_=bfv[:, WFC_B + 1024 * j:WFC_B + 1024 * (j + 1)]
                                  .rearrange("p (a m) -> p a m", a=DT))
                for c in range(2):
                    po = p4ps.tile([P, 512], F32, name="po")
                    for i in range(DT):
                        nc.tensor.matmul(po[:], ws[:, i, :], oT[:, i, 512 * c:512 * (c + 1)],
                                         start=(i == 0), stop=(i == DT - 1))
                    xr = p4x.tile([P, 512], F32, name="xr")
                    nc.sync.dma_start(out=xr, in_=xrow_c(j, 512 * c, 512))
                    nc.vector.tensor_add(x2[:, j, 512 * c:512 * (c + 1)], po[:], xr)

        # ---------------- Phase 5: LN2 -> xl2 bf16 ---------------------------
        xl2 = big.tile([P, DT, SQ], BF16, name="S3")

        with tc.tile_pool(name="p5s", bufs=2) as p5s, \
             tc.tile_pool(name="p5r", bufs=1) as p5r, \
             tc.tile_pool(name="p5ps", bufs=1, space="PSUM") as p5ps:
            sx2 = [p5ps.tile([1, 512], F32, name=f"sx2{c}") for c in range(2)]
            sq2 = [p5ps.tile([1, 512], F32, name=f"sq2{c}") for c in range(2)]
            for i in range(DT):
                for c in range(2):
                    cs = slice(512 * c, 512 * (c + 1))
                    sqc = p5s.tile([P, 512], F32R, name="sq2c_t")
                    nc.scalar.square(out=sqc, in_=x2[:, i, cs].bitcast(F32))
                    nc.tensor.matmul(sx2[c][:], ones_k[:], x2[:, i, cs],
                                     start=(i == 0), stop=(i == DT - 1))
                    nc.tensor.matmul(sq2[c][:], ones_k[:], sqc[:],
                                     start=(i == 0), stop=(i == DT - 1))
            mu2b = p5r.tile([P, SQ], F32, name="mu2b")
            rstd2b = p5r.tile([P, SQ], F32, name="rstd2b")
            for c in range(2):
                cs = slice(512 * c, 512 * (c + 1))
                mu_c = p5s.tile([1, 512], F32, name="mu2c")
                t_c = p5s.tile([1, 512], F32, name="t2c")
                msq_c = p5s.tile([1, 512], F32, name="msq2c")
                nc.scalar.mul(out=mu_c, in_=sx2[c][:], mul=1.0 / D)
                nc.vector.tensor_mul(t_c, mu_c, mu_c)
                nc.scalar.mul(out=msq_c, in_=sq2[c][:], mul=1.0 / D)
                nc.vector.tensor_sub(t_c, msq_c, t_c)
                nc.scalar.activation(out=t_c, in_=t_c, func=mybir.ActivationFunctionType.Sqrt,
                                     bias=eps_t, scale=1.0)
                nc.vector.reciprocal(out=t_c, in_=t_c)
                pb = p5ps.tile([P, 512], F32, name=f"sx2{c}")
                nc.tensor.matmul(pb[:], ones_m[:], mu_c[:], start=True, stop=True)
                nc.scalar.copy(out=mu2b[:, cs], in_=pb[:])
                pb2 = p5ps.tile([P, 512], F32, name=f"sq2{c}")
                nc.tensor.matmul(pb2[:], ones_m[:], t_c[:], start=True, stop=True)
                nc.scalar.copy(out=rstd2b[:, cs], in_=pb2[:])
            for c in range(2):
                cs = slice(512 * c, 512 * (c + 1))
                for i in range(DT):
                    t = p5s.tile([P, 512], F32, name="cen2")
                    nc.vector.tensor_sub(t, x2[:, i, cs].bitcast(F32), mu2b[:, cs])
                    nc.vector.tensor_mul(xl2[:, i, cs], t, rstd2b[:, cs])

        # ---------------- Phase 6: FFN + final residual ----------------------
        with tc.tile_pool(name="p6hh", bufs=1) as p6hh, \
             tc.tile_pool(name="p6w1", bufs=2) as p6w1, \
             tc.tile_pool(name="p6w2", bufs=2) as p6w2, \
             tc.tile_pool(name="p6o", bufs=2) as p6o, \
             tc.tile_pool(name="p6ps", bufs=3, space="PSUM") as p6ps:
            h_lo = big.tile([P, FT // 2, SQ], BF16, name="S1")
            h_hi = p6hh.tile([P, FT // 2, SQ], BF16, name="hhi")

            def hslice(t, cs):
                return (h_lo if t < FT // 2 else h_hi)[:, t % (FT // 2), cs]
            for ht in range(FT):
                w1s = p6w1.tile([P, DT, P], BF16, name="w1s")
                nc.sync.dma_start(out=w1s, in_=bfv[:, W1_B + 1024 * ht:W1_B + 1024 * (ht + 1)]
                                  .rearrange("p (a m) -> p a m", a=DT))
                for c in range(2):
                    pf = p6ps.tile([P, 512], F32, name="pf")
                    for i in range(DT):
                        nc.tensor.matmul(pf[:], w1s[:, i, :], xl2[:, i, 512 * c:512 * (c + 1)],
                                         start=(i == 0), stop=(i == DT - 1))
                    nc.scalar.activation(out=hslice(ht, slice(512 * c, 512 * (c + 1))), in_=pf[:],
                                         func=mybir.ActivationFunctionType.Relu)
            for j in range(DT):
                w2s = p6w2.tile([P, FT, P], BF16, name="w2s")
                nc.sync.dma_start(out=w2s, in_=bfv[:, W2_B + 4096 * j:W2_B + 4096 * (j + 1)]
                                  .rearrange("p (a m) -> p a m", a=FT))
                for c in range(2):
                    pf = p6ps.tile([P, 512], F32, name="pf")
                    for t in range(FT):
                        nc.tensor.matmul(pf[:], w2s[:, t, :], hslice(t, slice(512 * c, 512 * (c + 1))),
                                         start=(t == 0), stop=(t == FT - 1))
                    ob = p6o.tile([P, 512], F32, name="ob")
                    nc.vector.tensor_add(ob, pf[:], x2[:, j, 512 * c:512 * (c + 1)].bitcast(F32))
                    nc.sync.dma_start(out=outT[P * j:P * (j + 1), 512 * c:512 * (c + 1)], in_=ob)

        big.release()
        cst.release()

    nc.compile()
    return nc


def _get_nc():
    if "nc" not in _CACHE:
        _CACHE["nc"] = _build_nc()
    return _CACHE["nc"]


def _re_pam(w):
    ap, m = w.shape
    a = ap // P
    return np.ascontiguousarray(w.reshape(a, P, m).transpose(1, 0, 2).reshape(P, a * m))


def make_in_maps(inputs):
    x = np.asarray(inputs["x"], dtype=np.float32)

    def f32r_round(arr):
        u = np.ascontiguousarray(arr, dtype=np.float32).view(np.uint32)
        return ((u + 0x1000) & 0xFFFFE000).view(np.float32)

    wq3 = _re_pam(f32r_round(np.asarray(inputs["Wq"], dtype=np.float32) * 8.0)).reshape(P, DT, D)
    wk3 = _re_pam(f32r_round(np.asarray(inputs["Wk"], dtype=np.float32))).reshape(P, DT, D)
    wv3 = _re_pam(f32r_round(np.asarray(inputs["Wv"], dtype=np.float32))).reshape(P, DT, D)
    wv = np.concatenate([wv3[:, :, 0:512].reshape(P, -1), wv3[:, :, 512:1024].reshape(P, -1)], axis=1)
    wkq_parts = []
    for p in range(H // 2):
        wkq_parts.append(wk3[:, :, P * p:P * (p + 1)].reshape(P, -1))
        wkq_parts.append(wq3[:, :, P * p:P * (p + 1)].reshape(P, -1))
    wkq = np.concatenate(wkq_parts, axis=1)

    wfc3 = _re_pam(np.asarray(inputs["Wfc"], dtype=np.float32)).reshape(P, DT, D)
    wfc = np.concatenate([wfc3[:, :, P * j:P * (j + 1)].reshape(P, -1) for j in range(DT)], axis=1)
    w13 = _re_pam(np.asarray(inputs["W1"], dtype=np.float32)).reshape(P, DT, FF)
    w1 = np.concatenate([w13[:, :, P * t:P * (t + 1)].reshape(P, -1) for t in range(FT)], axis=1)
    w23 = _re_pam(np.asarray(inputs["W2"], dtype=np.float32)).reshape(P, FT, D)
    w2 = np.concatenate([w23[:, :, P * j:P * (j + 1)].reshape(P, -1) for j in range(DT)], axis=1)
    ident = np.eye(P, dtype=np.float32)
    bfall = np.concatenate([wfc, w1, w2, ident], axis=1).astype(ml_dtypes.bfloat16)
    bf_as_f32 = np.ascontiguousarray(bfall).view(np.float32)

    wblob = np.concatenate([wv, wkq, bf_as_f32,
                            np.ones((P, 1), dtype=np.float32)], axis=1).astype(np.float32)

    # x is consumed as f32r by the LN1 statistic matmuls on device; hardware
    # f32r matmuls require pre-rounded inputs (the BIR verifier enforces the
    # dtype chain, the rounding must happen here)
    x = f32r_round(x)
    in_maps = []
    for core in range(8):
        b, half = core // 2, core % 2
        xb = x[b]
        rot = np.concatenate([xb[SQ * half:SQ * (half + 1)], xb[SQ * (1 - half):SQ * (2 - half)]], axis=0)
        xTc = np.ascontiguousarray(rot.T)
        xpk = xTc.reshape(DT, P, S).transpose(1, 0, 2).reshape(P, DT * S)
        blob = np.concatenate([xpk, wblob], axis=1)
        in_maps.append({"blob": np.ascontiguousarray(blob)})
    return in_maps


def assemble_out(results, x_shape):
    out = np.empty(x_shape, dtype=np.float32)
    for core in range(8):
        b, half = core // 2, core % 2
        out[b, SQ * half:SQ * (half + 1), :] = results[core]["outT"].T
    return out


def kernel(**inputs):
    nc = _get_nc()
    in_maps = make_in_maps(inputs)
    res = run_bass_kernel_spmd(nc, in_maps, core_ids=list(range(8)))
    return assemble_out(res.results, np.asarray(inputs["x"]).shape)


# revision 4
# speedup vs baseline: 1.0042x; 1.0042x over previous
"""Trainium2 Bass kernel for nn_EncoderLayer_45621142618893 — v2.

Transformer encoder layer (D=1024, H=16 heads, S=2048, B=4), f32 in/out.
8 cores = (batch) x (sequence half); K/V over the full sequence per core
(duplicated across the pair — cheaper than collectives), Q/attention/FFN
for the core's own 1024 tokens. Zero cross-core communication.

v2 vs baseline:
  - ALL inputs packed into one dram tensor (blob) per core.
  - xl (LN1 output) kept SBUF-resident in f32r; no DRAM scratch spill.
  - LN statistic sums + broadcasts done as f32r matmuls (4x cheaper on PE
    than f32).
  - K/Q projections fused per head-pair with attention: only a [128,2048]
    K-pair and [128,1024] Q-pair tile live at a time instead of the full
    K^T/Q^T (saves ~12MB SBUF -> enables xl residency).
  - V projected in bulk (token-major, bf16) from resident xl.
  - FFN W2 slabs double-buffered.

Blob layout [128, 77888] f32 cols:
  0:16384        xT row-blocks [128,2048] x8 (block i at 2048*i)
  16384:24576    wv f32r, V-pass order: two cc-slabs [128, 8, 512] flat
  24576:40960    per-pair [wk_p | wq_p] f32r: pair p at 24576+2048p,
                 each [128, 8, 128] flat 1024 cols (wq pre-scaled by 8)
  40960:77888    bf16 region (bitcast), bf16 col offsets:
      0:8192         wfc slabs j=0..8 of [128, 8, 128] flat
      8192:40960     w1 slabs ht=0..32 of [128, 8, 128] flat
      40960:73728    w2 slabs j=0..8 of [128, 32, 128] flat
      73728:73856    identity [128, 128]
"""

import sys

sys.path.insert(0, "/opt/trn_rl_repo")

import numpy as np
import ml_dtypes

import concourse.bacc as bacc
import concourse.tile as tile
from concourse import mybir
from concourse.bass_utils import run_bass_kernel_spmd

P = 128
D = 1024
S = 2048
SQ = 1024
H = 16
C = 64
FF = 4096
DT = D // P
TT = S // P
FT = FF // P

F32 = mybir.dt.float32
F32R = mybir.dt.float32r
BF16 = mybir.dt.bfloat16

XOFF = 0
WVOFF = 16384
WKQOFF = 24576
BFOFF = 40960
ONE_OFF = 77888
NCOL = 77889
WFC_B = 0
W1_B = 8192
W2_B = 40960
ID_B = 73728

_CACHE = {}


def _build_nc():
    import os
    dbg_stage = os.environ.get("KV2_DEBUG_STAGE", "")
    nc = bacc.Bacc("TRN2", target_bir_lowering=False, debug=False, num_devices=8)

    blob = nc.dram_tensor("blob", [P, NCOL], F32, kind="ExternalInput")
    outT = nc.dram_tensor("outT", [D, SQ], F32, kind="ExternalOutput")
    dbg = None
    if dbg_stage in ("xl", "x2"):
        dbg = nc.dram_tensor("dbg", [P, DT * S if dbg_stage == "xl" else DT * SQ],
                             F32, kind="ExternalOutput")
    elif dbg_stage in ("v", "oT"):
        dbg = nc.dram_tensor("dbg", [P, TT * D if dbg_stage == "v" else DT * SQ],
                             BF16, kind="ExternalOutput")
    elif dbg_stage == "kq":
        dbg = nc.dram_tensor("dbg", [P, S + SQ], F32, kind="ExternalOutput")

    def xrow(i):
        return blob[:, XOFF + S * i: XOFF + S * (i + 1)]

    def xrow_c(j, c0, w):
        return blob[:, XOFF + S * j + c0: XOFF + S * j + c0 + w]

    bfv = blob[:, BFOFF:ONE_OFF].bitcast(BF16)

    with tile.TileContext(nc) as tc:
        cst = tc.alloc_tile_pool(name="cst", bufs=1)
        idt = cst.tile([P, P], BF16, name="idt")
        nc.sync.dma_start(out=idt, in_=bfv[:, ID_B:ID_B + P])
        ones_k = cst.tile([P, 1], F32R, name="ones_k")
        nc.sync.dma_start(out=ones_k, in_=blob[:, ONE_OFF:ONE_OFF + 1].bitcast(F32R))
        ones_m = cst.tile([1, P], F32, name="ones_m")
        nc.vector.memset(ones_m, 1.0)
        eps_t = cst.tile([1, 1], F32, name="eps_t")
        nc.vector.memset(eps_t, 1e-5)

        big = tc.alloc_tile_pool(name="big", bufs=1)
        xl = big.tile([P, DT, S], F32R, name="S1")     # LN1 out, f32r, resident
        v_t = big.tile([P, TT, D], BF16, name="S2")    # V token-major

        # ---------------- Phase 1: LN1 stats + center -> xl ------------------
        with tc.tile_pool(name="p1s", bufs=7) as p1s, \
             tc.tile_pool(name="p1q", bufs=2) as p1q, \
             tc.tile_pool(name="p1r", bufs=1) as p1r, \
             tc.tile_pool(name="p1ps", bufs=1, space="PSUM") as p1ps:
            sx = [p1ps.tile([1, 512], F32, name=f"sx{c}") for c in range(4)]
            sq = [p1ps.tile([1, 512], F32, name=f"sq{c}") for c in range(4)]
            xts = []
            for i in range(DT):
                xt = p1s.tile([P, S], F32R, name="xt")
                xts.append(xt)
                nc.sync.dma_start(out=xt, in_=xrow(i).bitcast(F32R))
                for c in range(4):
                    cs = slice(512 * c, 512 * (c + 1))
                    sqc = p1q.tile([P, 512], F32R, name="sqc")
                    nc.scalar.square(out=sqc, in_=xt[:, cs].bitcast(F32))
                    nc.tensor.matmul(sx[c][:], ones_k[:], xt[:, cs],
                                     start=(i == 0), stop=(i == DT - 1))
                    nc.tensor.matmul(sq[c][:], ones_k[:], sqc[:],
                                     start=(i == 0), stop=(i == DT - 1))
            # 8 tiles in a 7-slot ring: tile 7 evicted block 0's slot, so
            # block 0 must be re-streamed during centering
            xts[0] = None
            mub = p1r.tile([P, S], F32, name="mub")
            rstdb = p1r.tile([P, S], F32, name="rstdb")
            for c in range(4):
                cs = slice(512 * c, 512 * (c + 1))
                mu_c = p1q.tile([1, 512], F32, name="mu_c")
                t_c = p1q.tile([1, 512], F32, name="t_c")
                nc.scalar.mul(out=mu_c, in_=sx[c][:], mul=1.0 / D)
                nc.vector.tensor_mul(t_c, mu_c, mu_c)
                msq_c = p1q.tile([1, 512], F32, name="msq_c")
                nc.scalar.mul(out=msq_c, in_=sq[c][:], mul=1.0 / D)
                nc.vector.tensor_sub(t_c, msq_c, t_c)
                nc.scalar.activation(out=t_c, in_=t_c, func=mybir.ActivationFunctionType.Sqrt,
                                     bias=eps_t, scale=1.0)
                nc.vector.reciprocal(out=t_c, in_=t_c)
                pb = p1ps.tile([P, 512], F32, name=f"sx{c}")
                nc.tensor.matmul(pb[:], ones_m[:], mu_c[:], start=True, stop=True)
                nc.scalar.copy(out=mub[:, cs], in_=pb[:])
                pb2 = p1ps.tile([P, 512], F32, name=f"sq{c}")
                nc.tensor.matmul(pb2[:], ones_m[:], t_c[:], start=True, stop=True)
                nc.scalar.copy(out=rstdb[:, cs], in_=pb2[:])
            # center token-chunk-major so phase 2 can start on early chunks
            for c in range(4):
                cs = slice(512 * c, 512 * (c + 1))
                for i in range(DT):
                    xtc = p1q.tile([P, 512], F32, name="xtc")
                    if xts[i] is None:
                        nc.sync.dma_start(out=xtc, in_=xrow_c(i, 512 * c, 512))
                        nc.vector.tensor_sub(xtc, xtc, mub[:, cs])
                    else:
                        nc.vector.tensor_sub(xtc, xts[i][:, cs].bitcast(F32), mub[:, cs])
                    nc.vector.tensor_mul(xl[:, i, cs], xtc, rstdb[:, cs])

        if dbg_stage == "xl":
            nc.sync.dma_start(out=dbg[:, :], in_=xl.bitcast(F32).rearrange("p a t -> p (a t)"))

        # ---------------- Phase 2: V (bulk, token-major bf16) ----------------
        with tc.tile_pool(name="p2w", bufs=2) as p2w, \
             tc.tile_pool(name="p2ps", bufs=2, space="PSUM") as p2ps:
            for cc in range(2):
                wvs = p2w.tile([P, DT, 512], F32R, name="wvs")
                nc.sync.dma_start(out=wvs, in_=blob[:, WVOFF + 4096 * cc:WVOFF + 4096 * (cc + 1)]
                                  .bitcast(F32R).rearrange("p (a m) -> p a m", a=DT))
                for tt in range(TT):
                    pv = p2ps.tile([P, 512], F32, name="pv")
                    for i in range(DT):
                        nc.tensor.matmul(pv[:], xl[:, i, P * tt:P * (tt + 1)], wvs[:, i, :],
                                         start=(i == 0), stop=(i == DT - 1))
                    nc.scalar.copy(out=v_t[:, tt, 512 * cc:512 * (cc + 1)], in_=pv[:])

        if dbg_stage == "v":
            nc.sync.dma_start(out=dbg[:, :], in_=v_t.rearrange("p a t -> p (a t)"))

        # ------- Phase 3: per head-pair K/Q projection + attention -----------
        oT = big.tile([P, DT, SQ], BF16, name="S3")

        with tc.tile_pool(name="p3w", bufs=2) as p3w, \
             tc.tile_pool(name="p3kq", bufs=3) as p3kq, \
             tc.tile_pool(name="p3a", bufs=3) as p3a, \
             tc.tile_pool(name="p3t", bufs=3) as p3t, \
             tc.tile_pool(name="p3r", bufs=8) as p3r, \
             tc.tile_pool(name="p3sc", bufs=5, space="PSUM") as p3sc, \
             tc.tile_pool(name="p3tp", bufs=2, space="PSUM") as p3tp, \
             tc.tile_pool(name="p3ov", bufs=1, space="PSUM") as p3ov:
            for p in range(H // 2):
                wkq = p3w.tile([P, 2, DT, P], F32R, name="wkq")
                nc.sync.dma_start(out=wkq, in_=blob[:, WKQOFF + 2048 * p:WKQOFF + 2048 * (p + 1)]
                                  .bitcast(F32R).rearrange("p (w a m) -> p w a m", w=2, a=DT))
                k_p = p3kq.tile([P, S], F32R, name="k_p")
                for c in range(4):
                    pk = p3sc.tile([P, 512], F32, name="sc")
                    for i in range(DT):
                        nc.tensor.matmul(pk[:], wkq[:, 0, i, :], xl[:, i, 512 * c:512 * (c + 1)],
                                         start=(i == 0), stop=(i == DT - 1))
                    nc.scalar.copy(out=k_p[:, 512 * c:512 * (c + 1)], in_=pk[:])
                q_p = p3kq.tile([P, SQ], F32R, name="q_p")
                for c in range(2):
                    pk = p3sc.tile([P, 512], F32, name="sc")
                    for i in range(DT):
                        nc.tensor.matmul(pk[:], wkq[:, 1, i, :], xl[:, i, 512 * c:512 * (c + 1)],
                                         start=(i == 0), stop=(i == DT - 1))
                    nc.scalar.copy(out=q_p[:, 512 * c:512 * (c + 1)], in_=pk[:])
                if dbg_stage == "kq" and p == 0:
                    nc.sync.dma_start(out=dbg[:, 0:S], in_=k_p[:].bitcast(F32))
                    nc.sync.dma_start(out=dbg[:, S:S + SQ], in_=q_p[:].bitcast(F32))

                for g in range(4):
                    aTs = []
                    for hh in range(2):
                        base = 64 * hh
                        aT = p3t.tile([P, TT, 256], BF16, name="aT")
                        aTs.append(aT)
                        for q2 in range(2):
                            qt = 2 * g + q2
                            at = p3a.tile([P, S], BF16, name="at")
                            mn4 = p3r.tile([P, 4], F32, name="mn4")
                            mmin = p3r.tile([P, 1], F32, name="mmin")
                            f4 = p3r.tile([P, 4], F32, name="f4")
                            zp = p3r.tile([P, 4], F32, name="zp")
                            zz = p3r.tile([P, 4], F32, name="zz")
                            zs = p3r.tile([P, 1], F32, name="zs")
                            rr = p3r.tile([P, 1], F32, name="rr")
                            # streaming softmax: per-512-chunk max+exp, exact
                            # rescale combine at the end (flash-style)
                            for kc in range(4):
                                sck = p3sc.tile([P, 512], F32, name="sc")
                                nc.tensor.matmul(
                                    sck[:],
                                    q_p[base:base + 64, P * qt:P * (qt + 1)],
                                    k_p[base:base + 64, 512 * kc:512 * (kc + 1)],
                                    start=True, stop=True)
                                nc.vector.reduce_max(out=mn4[:, kc:kc + 1], in_=sck[:],
                                                     axis=mybir.AxisListType.X, negate=True)
                                nc.scalar.activation(
                                    out=at[:, 512 * kc:512 * (kc + 1)], in_=sck[:],
                                    func=mybir.ActivationFunctionType.Exp,
                                    bias=mn4[:, kc:kc + 1], scale=1.0,
                                    accum_out=zp[:, kc:kc + 1])
                            # mn4 = -m_kc; global -m = min(mn4); f_kc = exp(m_kc - m)
                            nc.vector.tensor_reduce(out=mmin, in_=mn4, axis=mybir.AxisListType.X,
                                                    op=mybir.AluOpType.min)
                            nc.scalar.activation(out=f4, in_=mn4,
                                                 func=mybir.ActivationFunctionType.Exp,
                                                 bias=mmin, scale=-1.0)
                            nc.vector.tensor_mul(zz, f4, zp)
                            nc.vector.reduce_sum(out=zs, in_=zz, axis=mybir.AxisListType.X)
                            nc.vector.reciprocal(out=rr, in_=zs)
                            nc.vector.tensor_scalar_mul(out=f4, in0=f4, scalar1=rr)
                            for kc in range(4):
                                nc.gpsimd.tensor_scalar_mul(
                                    out=at[:, 512 * kc:512 * (kc + 1)],
                                    in0=at[:, 512 * kc:512 * (kc + 1)],
                                    scalar1=f4[:, kc:kc + 1])
                            for kb in range(4):
                                tp = p3tp.tile([P, 512], BF16, name="tp")
                                for k4 in range(4):
                                    ki = 4 * kb + k4
                                    nc.tensor.transpose(tp[:, P * k4:P * (k4 + 1)],
                                                        at[:, P * ki:P * (ki + 1)], idt[:])
                                dst = aT[:, 4 * kb:4 * (kb + 1), P * q2:P * (q2 + 1)]
                                src = tp.rearrange("p (a b) -> p a b", a=4)
                                if kb % 2 == 0:
                                    nc.vector.tensor_copy(out=dst, in_=src)
                                else:
                                    nc.scalar.copy(out=dst, in_=src)
                    po = p3ov.tile([P, 256], F32, name="po")
                    for hh in range(2):
                        h64 = 64 * (2 * p + hh)
                        for kt in range(TT):
                            nc.tensor.matmul(po[64 * hh:64 * (hh + 1), :], v_t[:, kt, h64:h64 + 64],
                                             aTs[hh][:, kt, :], start=(kt == 0), stop=(kt == TT - 1))
                    nc.scalar.copy(out=oT[:, p, 256 * g:256 * (g + 1)], in_=po[:])

        if dbg_stage == "oT":
            nc.sync.dma_start(out=dbg[:, :], in_=oT.rearrange("p a t -> p (a t)"))

        # ---------------- Phase 4: O-projection + residual -> x2 -------------
        # f32r so LN2's statistic matmuls can consume it directly (the BIR
        # verifier requires f32r-matmul inputs to be produced as f32r)
        x2 = big.tile([P, DT, SQ], F32R, name="S2")

        with tc.tile_pool(name="p4w", bufs=2) as p4w, \
             tc.tile_pool(name="p4x", bufs=2) as p4x, \
             tc.tile_pool(name="p4ps", bufs=3, space="PSUM") as p4ps:
            for j in range(DT):
                ws = p4w.tile([P, DT, P], BF16, name="ws")
                nc.sync.dma_start(out=ws, in_=bfv[:, WFC_B + 1024 * j:WFC_B + 1024 * (j + 1)]
                                  .rearrange("p (a m) -> p a m", a=DT))
                for c in range(2):
                    po = p4ps.tile([P, 512], F32, name="po")
                    for i in range(DT):
                        nc.tensor.matmul(po[:], ws[:, i, :], oT[:, i, 512 * c:512 * (c + 1)],
                                         start=(i == 0), stop=(i == DT - 1))
                    xr = p4x.tile([P, 512], F32, name="xr")
                    nc.sync.dma_start(out=xr, in_=xrow_c(j, 512 * c, 512))
                    nc.vector.tensor_add(x2[:, j, 512 * c:512 * (c + 1)], po[:], xr)

        # ---------------- Phase 5: LN2 -> xl2 bf16 ---------------------------
        xl2 = big.tile([P, DT, SQ], BF16, name="S3")

        with tc.tile_pool(name="p5s", bufs=2) as p5s, \
             tc.tile_pool(name="p5r", bufs=1) as p5r, \
             tc.tile_pool(name="p5ps", bufs=1, space="PSUM") as p5ps:
            sx2 = [p5ps.tile([1, 512], F32, name=f"sx2{c}") for c in range(2)]
            sq2 = [p5ps.tile([1, 512], F32, name=f"sq2{c}") for c in range(2)]
            for i in range(DT):
                for c in range(2):
                    cs = slice(512 * c, 512 * (c + 1))
                    sqc = p5s.tile([P, 512], F32R, name="sq2c_t")
                    nc.scalar.square(out=sqc, in_=x2[:, i, cs].bitcast(F32))
                    nc.tensor.matmul(sx2[c][:], ones_k[:], x2[:, i, cs],
                                     start=(i == 0), stop=(i == DT - 1))
                    nc.tensor.matmul(sq2[c][:], ones_k[:], sqc[:],
                                     start=(i == 0), stop=(i == DT - 1))
            mu2b = p5r.tile([P, SQ], F32, name="mu2b")
            rstd2b = p5r.tile([P, SQ], F32, name="rstd2b")
            for c in range(2):
                cs = slice(512 * c, 512 * (c + 1))
                mu_c = p5s.tile([1, 512], F32, name="mu2c")
                t_c = p5s.tile([1, 512], F32, name="t2c")
                msq_c = p5s.tile([1, 512], F32, name="msq2c")
                nc.scalar.mul(out=mu_c, in_=sx2[c][:], mul=1.0 / D)
                nc.vector.tensor_mul(t_c, mu_c, mu_c)
                nc.scalar.mul(out=msq_c, in_=sq2[c][:], mul=1.0 / D)
                nc.vector.tensor_sub(t_c, msq_c, t_c)
                nc.scalar.activation(out=t_c, in_=t_c, func=mybir.ActivationFunctionType.Sqrt,
                                     bias=eps_t, scale=1.0)
                nc.vector.reciprocal(out=t_c, in_=t_c)
                pb = p5ps.tile([P, 512], F32, name=f"sx2{c}")
                nc.tensor.matmul(pb[:], ones_m[:], mu_c[:], start=True, stop=True)
                nc.scalar.copy(out=mu2b[:, cs], in_=pb[:])
                pb2 = p5ps.tile([P, 512], F32, name=f"sq2{c}")
                nc.tensor.matmul(pb2[:], ones_m[:], t_c[:], start=True, stop=True)
                nc.scalar.copy(out=rstd2b[:, cs], in_=pb2[:])
            for c in range(2):
                cs = slice(512 * c, 512 * (c + 1))
                for i in range(DT):
                    t = p5s.tile([P, 512], F32, name="cen2")
                    nc.vector.tensor_sub(t, x2[:, i, cs].bitcast(F32), mu2b[:, cs])
                    nc.vector.tensor_mul(xl2[:, i, cs], t, rstd2b[:, cs])

        # ---------------- Phase 6: FFN + final residual ----------------------
        with tc.tile_pool(name="p6hh", bufs=1) as p6hh, \
             tc.tile_pool(name="p6w1", bufs=2) as p6w1, \
             tc.tile_pool(name="p6w2", bufs=2) as p6w2, \
             tc.tile_pool(name="p6o", bufs=2) as p6o, \
             tc.tile_pool(name="p6ps", bufs=3, space="PSUM") as p6ps:
            h_lo = big.tile([P, FT // 2, SQ], BF16, name="S1")
            h_hi = p6hh.tile([P, FT // 2, SQ], BF16, name="hhi")

            def hslice(t, cs):
                return (h_lo if t < FT // 2 else h_hi)[:, t % (FT // 2), cs]
            for ht in range(FT):
                w1s = p6w1.tile([P, DT, P], BF16, name="w1s")
                nc.sync.dma_start(out=w1s, in_=bfv[:, W1_B + 1024 * ht:W1_B + 1024 * (ht + 1)]
                                  .rearrange("p (a m) -> p a m", a=DT))
                for c in range(2):
                    pf = p6ps.tile([P, 512], F32, name="pf")
                    for i in range(DT):
                        nc.tensor.matmul(pf[:], w1s[:, i, :], xl2[:, i, 512 * c:512 * (c + 1)],
                                         start=(i == 0), stop=(i == DT - 1))
                    nc.scalar.activation(out=hslice(ht, slice(512 * c, 512 * (c + 1))), in_=pf[:],
                                         func=mybir.ActivationFunctionType.Relu)
            for j in range(DT):
                w2s = p6w2.tile([P, FT, P], BF16, name="w2s")
                nc.sync.dma_start(out=w2s, in_=bfv[:, W2_B + 4096 * j:W2_B + 4096 * (j + 1)]
                                  .rearrange("p (a m) -> p a m", a=FT))
                for c in range(2):
                    pf = p6ps.tile([P, 512], F32, name="pf")
                    for t in range(FT):
                        nc.tensor.matmul(pf[:], w2s[:, t, :], hslice(t, slice(512 * c, 512 * (c + 1))),
                                         start=(t == 0), stop=(t == FT - 1))
                    ob = p6o.tile([P, 512], F32, name="ob")
                    nc.vector.tensor_add(ob, pf[:], x2[:, j, 512 * c:512 * (c + 1)].bitcast(F32))
                    nc.sync.dma_start(out=outT[P * j:P * (j + 1), 512 * c:512 * (c + 1)], in_=ob)

        big.release()
        cst.release()

    nc.compile()
    return nc


def _get_nc():
    if "nc" not in _CACHE:
        _CACHE["nc"] = _build_nc()
    return _CACHE["nc"]


def _re_pam(w):
    ap, m = w.shape
    a = ap // P
    return np.ascontiguousarray(w.reshape(a, P, m).transpose(1, 0, 2).reshape(P, a * m))


def make_in_maps(inputs):
    x = np.asarray(inputs["x"], dtype=np.float32)

    def f32r_round(arr):
        u = np.ascontiguousarray(arr, dtype=np.float32).view(np.uint32)
        return ((u + 0x1000) & 0xFFFFE000).view(np.float32)

    wq3 = _re_pam(f32r_round(np.asarray(inputs["Wq"], dtype=np.float32) * 8.0)).reshape(P, DT, D)
    wk3 = _re_pam(f32r_round(np.asarray(inputs["Wk"], dtype=np.float32))).reshape(P, DT, D)
    wv3 = _re_pam(f32r_round(np.asarray(inputs["Wv"], dtype=np.float32))).reshape(P, DT, D)
    wv = np.concatenate([wv3[:, :, 0:512].reshape(P, -1), wv3[:, :, 512:1024].reshape(P, -1)], axis=1)
    wkq_parts = []
    for p in range(H // 2):
        wkq_parts.append(wk3[:, :, P * p:P * (p + 1)].reshape(P, -1))
        wkq_parts.append(wq3[:, :, P * p:P * (p + 1)].reshape(P, -1))
    wkq = np.concatenate(wkq_parts, axis=1)

    wfc3 = _re_pam(np.asarray(inputs["Wfc"], dtype=np.float32)).reshape(P, DT, D)
    wfc = np.concatenate([wfc3[:, :, P * j:P * (j + 1)].reshape(P, -1) for j in range(DT)], axis=1)
    w13 = _re_pam(np.asarray(inputs["W1"], dtype=np.float32)).reshape(P, DT, FF)
    w1 = np.concatenate([w13[:, :, P * t:P * (t + 1)].reshape(P, -1) for t in range(FT)], axis=1)
    w23 = _re_pam(np.asarray(inputs["W2"], dtype=np.float32)).reshape(P, FT, D)
    w2 = np.concatenate([w23[:, :, P * j:P * (j + 1)].reshape(P, -1) for j in range(DT)], axis=1)
    ident = np.eye(P, dtype=np.float32)
    bfall = np.concatenate([wfc, w1, w2, ident], axis=1).astype(ml_dtypes.bfloat16)
    bf_as_f32 = np.ascontiguousarray(bfall).view(np.float32)

    wblob = np.concatenate([wv, wkq, bf_as_f32,
                            np.ones((P, 1), dtype=np.float32)], axis=1).astype(np.float32)

    # x is consumed as f32r by the LN1 statistic matmuls on device; hardware
    # f32r matmuls require pre-rounded inputs (the BIR verifier enforces the
    # dtype chain, the rounding must happen here)
    x = f32r_round(x)
    in_maps = []
    for core in range(8):
        b, half = core // 2, core % 2
        xb = x[b]
        rot = np.concatenate([xb[SQ * half:SQ * (half + 1)], xb[SQ * (1 - half):SQ * (2 - half)]], axis=0)
        xTc = np.ascontiguousarray(rot.T)
        xpk = xTc.reshape(DT, P, S).transpose(1, 0, 2).reshape(P, DT * S)
        blob = np.concatenate([xpk, wblob], axis=1)
        in_maps.append({"blob": np.ascontiguousarray(blob)})
    return in_maps


def assemble_out(results, x_shape):
    out = np.empty(x_shape, dtype=np.float32)
    for core in range(8):
        b, half = core // 2, core % 2
        out[b, SQ * half:SQ * (half + 1), :] = results[core]["outT"].T
    return out


def kernel(**inputs):
    nc = _get_nc()
    in_maps = make_in_maps(inputs)
    res = run_bass_kernel_spmd(nc, in_maps, core_ids=list(range(8)))
    return assemble_out(res.results, np.asarray(inputs["x"]).shape)
